# revision 10
# baseline (speedup 1.0000x reference)
"""ListMLE-with-tail loss kernel for Trainium2 (Bass/Tile), 8-core data-parallel.

Full-input contract: kernel(output[1024,50000] f32, target[1024] i32,
tails[1024,50] i32, tail_len[1024] i32) -> neg_like[1024] f32.

Sharding: batch rows split 128 per core (one row per SBUF partition).
Per core the dominant work - streaming the [128, 50000] row-slice through
the scalar engine's exp with fused per-chunk row-sum accumulation - runs
on device; the tail term (cumsum over <=50 exps + logs) also runs on
device in f32.

Two deliberate host-side preprocessing steps (both validated against the
f32 reference, gate is 2e-2):
- x is downcast to fp16 before upload: halves the HBM stream from 25.6MB
  to 12.8MB per core (measured rel err 6.3e-5).
- The 51 per-row score lookups (target + reversed tails; 13KB per core)
  are gathered on host from the same fp16 array the device streams, so
  the arithmetic is bit-identical to a device-side gather.  All three
  device gather mechanisms were measured on HW and share one serial Q7
  descriptor-generation path at ~10ns/element: 51 indirect DMAs = 85us,
  one dma_gather of 6656 window indices = 56us - either would dominate
  the ~58us exp stream, while the host gather is 0.1% of the input.

Scheduling notes (from NTFF traces of earlier revisions):
- Chunk-0's DMA is issued before the small uploads so the activation
  stream starts ASAP.
- log(total) is folded into the tail Ln activation as an extra input
  column, so the tail costs one Exp + one Ln (one act-table switch).
"""

import functools

import numpy as np

import concourse.bass as bass
import concourse.bacc as bacc
import concourse.tile as tile
from concourse import mybir
from concourse.bass_utils import run_bass_kernel_spmd

B = 1024
V = 50000
T = 50
M = 8            # cores
P = B // M       # 128 rows per core = SBUF partitions
# Ramped chunk schedule: small first chunk so the scalar engine starts
# ~2us earlier (the ~2us DMA completion receipt dominates small chunks),
# larger steady-state chunks to amortize the per-ACTIVATE overhead.
CHUNKS = [2048, 3072, 4096, 6144, 8192, 8192, 9128, 9128]
assert sum(CHUNKS) == V
G = T + 1        # gathered scores per row: [target, reversed tails]

F16 = mybir.dt.float16
F32 = mybir.dt.float32


def _build_program() -> bass.Bass:
    nc = bacc.Bacc()
    x = nc.dram_tensor("x", [P, V], F16, kind="ExternalInput")
    sg = nc.dram_tensor("sg", [P, G], F16, kind="ExternalInput")
    maskr = nc.dram_tensor("maskr", [P, T], F32, kind="ExternalInput")
    loss = nc.dram_tensor("loss", [P, 1], F32, kind="ExternalOutput")

    with tile.TileContext(nc) as tc:
        with (
            tc.tile_pool(name="inp", bufs=1) as inp,
            tc.tile_pool(name="small", bufs=1) as small,
        ):
            # All chunk tiles are distinct (100KB/partition total) so the DMA
            # queue can run arbitrarily far ahead of the activation stream.
            # Chunk-0 DMA first: the scalar engine starts the exp stream
            # earlier than if the small uploads went first.
            xts = []
            off = 0
            for csz in CHUNKS:
                xt = inp.tile([P, csz], F16)
                nc.sync.dma_start(out=xt[:], in_=x[:, off:off + csz])
                xts.append(xt)
                off += csz

            # Small uploads go after every chunk issue: they are not needed
            # until the tail (~58us), while chunk N+1's issue latency is on
            # the activation stream's critical path early on.
            sg_t = small.tile([P, G], F16)
            nc.sync.dma_start(out=sg_t[:], in_=sg[:])
            maskr_t = small.tile([P, T], F32)
            nc.sync.dma_start(out=maskr_t[:], in_=maskr[:])

            # Funnel DMA-produced tiles through one DVE copy each so no
            # downstream instruction needs >1 cross-engine sync wait (the
            # TensorTensor encoding carries a single wait slot).  sg2 also
            # upcasts the gathered scores to f32 for the tail arithmetic.
            maskr2 = small.tile([P, T], F32)
            nc.vector.tensor_copy(out=maskr2[:], in_=maskr_t[:])
            sg2 = small.tile([P, G], F32)
            nc.vector.tensor_copy(out=sg2[:], in_=sg_t[:])

            # Main stream: total_exp[p] = sum_v exp(x[p, v]), chunked.  The
            # exp output itself is dead; one max-size scratch tile is reused
            # (consecutive ACTIVATEs are serial on Scalar anyway).
            sums = small.tile([P, len(CHUNKS)], F32)
            et = small.tile([P, max(CHUNKS)], F16)
            for i, csz in enumerate(CHUNKS):
                nc.scalar.activation(
                    out=et[:, 0:csz],
                    in_=xts[i][:],
                    func=mybir.ActivationFunctionType.Exp,
                    accum_out=sums[:, i:i + 1],
                )
            total = small.tile([P, 1], F32)
            nc.vector.reduce_sum(out=total[:], in_=sums[:], axis=mybir.AxisListType.X)

            # Tail term, all [P, <=52] f32 ops.  e_all has no artificial
            # ordering pin: its input is ready early, so wherever the
            # scheduler slots it on Scalar it reuses the loaded Exp table.
            e_all = small.tile([P, G], F32)
            nc.scalar.activation(
                out=e_all[:], in_=sg_t[:], func=mybir.ActivationFunctionType.Exp
            )
            es = small.tile([P, T], F32)
            nc.vector.tensor_mul(out=es[:], in0=e_all[:, 1:G], in1=maskr2[:])
            # lgin[:, 0:T] = cumsum(es) along t (== reference's cumsum of the
            # flipped masked exps); lgin[:, T] = total - others, so a single
            # Ln yields both the tail logs and log(total).
            lgin = small.tile([P, T + 1], F32)
            nc.vector.tensor_tensor_scan(
                out=lgin[:, 0:T],
                data0=es[:],
                data1=es[:],
                initial=0.0,
                op0=mybir.AluOpType.add,
                op1=mybir.AluOpType.bypass,
            )
            # others = total - exp(target_score) - sum(es); sum(es) = lgin[:, T-1]
            others = small.tile([P, 1], F32)
            nc.vector.tensor_scalar(
                out=others[:],
                in0=total[:],
                scalar1=e_all[:, 0:1],
                scalar2=lgin[:, T - 1:T],
                op0=mybir.AluOpType.subtract,
                op1=mybir.AluOpType.subtract,
            )
            nc.vector.tensor_scalar(
                out=lgin[:, T:T + 1],
                in0=total[:],
                scalar1=others[:],
                scalar2=None,
                op0=mybir.AluOpType.subtract,
            )
            # lg[:, t<T] = log(c_t + others); lg[:, T] = log(total)
            lg = small.tile([P, T + 1], F32)
            nc.scalar.activation(
                out=lg[:],
                in_=lgin[:],
                func=mybir.ActivationFunctionType.Ln,
                bias=others[:],
            )
            wl = small.tile([P, T], F32)
            nc.vector.tensor_mul(out=wl[:], in0=lg[:, 0:T], in1=maskr2[:])
            below = small.tile([P, 1], F32)
            nc.vector.reduce_sum(out=below[:], in_=wl[:], axis=mybir.AxisListType.X)
            sm = small.tile([P, T], F32)
            nc.vector.tensor_mul(out=sm[:], in0=sg2[:, 1:G], in1=maskr2[:])
            above = small.tile([P, 1], F32)
            nc.vector.reduce_sum(out=above[:], in_=sm[:], axis=mybir.AxisListType.X)

            # loss = -(target_score - log(total) + above - below)
            t1 = small.tile([P, 1], F32)
            nc.vector.tensor_scalar(
                out=t1[:],
                in0=lg[:, T:T + 1],
                scalar1=sg2[:, 0:1],
                scalar2=above[:],
                op0=mybir.AluOpType.subtract,
                op1=mybir.AluOpType.subtract,
            )
            res = small.tile([P, 1], F32)
            nc.vector.tensor_add(out=res[:], in0=t1[:], in1=below[:])
            nc.sync.dma_start(out=loss[:], in_=res[:])
    nc.finalize()  # runs the bacc passes (sync-wait splitting etc.)
    return nc


@functools.cache
def _program() -> bass.Bass:
    return _build_program()


def _prep_core_inputs(x16, target, tails, tail_len, core):
    r0 = core * P
    x = x16[r0:r0 + P]
    tgt = target[r0:r0 + P].astype(np.int64)
    tls = tails[r0:r0 + P].astype(np.int64)
    tln = tail_len[r0:r0 + P].astype(np.int64)

    # sg[p, 0] = x[p, target[p]]; sg[p, 1+t] = x[p, tails[p, T-1-t]]
    idx = np.concatenate([tgt[:, None], tls[:, ::-1]], axis=1)
    sg = np.take_along_axis(x, idx, axis=1)

    # maskr[r, t] = 1 iff reversed-tail position t is valid: (T-1-t) < tail_len[r]
    tpos = np.arange(T - 1, -1, -1, dtype=np.int64)[None, :]
    maskr = (tpos < tln[:, None]).astype(np.float32)
    return {
        "x": x,
        "sg": np.ascontiguousarray(sg),
        "maskr": np.ascontiguousarray(maskr),
    }


TRACE = False  # set by test.py for profiling runs; harness leaves it False


def kernel(output, target, tails, tail_len):
    output = np.asarray(output, dtype=np.float32)
    target = np.asarray(target)
    tails = np.asarray(tails)
    tail_len = np.asarray(tail_len)

    x16 = np.ascontiguousarray(output.astype(np.float16))
    in_maps = [
        _prep_core_inputs(x16, target, tails, tail_len, core) for core in range(M)
    ]
    out = run_bass_kernel_spmd(
        _program(), in_maps, core_ids=list(range(M)), trace=TRACE
    )
    global last_result
    last_result = out
    return np.concatenate(
        [r["loss"].reshape(P).astype(np.float32) for r in out.results]
    )


last_result = None


# revision 13
# speedup vs baseline: 1.8715x; 1.8715x over previous
"""ListMLE-with-tail loss kernel for Trainium2 (Bass/Tile), 8-core data-parallel.

Full-input contract: kernel(output[1024,50000] f32, target[1024] i32,
tails[1024,50] i32, tail_len[1024] i32) -> neg_like[1024] f32.

Sharding: batch rows split 128 per core (one row per SBUF partition).
Per core the dominant work - streaming the [128, 50000] row-slice through
the scalar engine's exp with fused per-chunk row-sum accumulation - runs
on device; the tail term (cumsum over <=50 exps + logs) also runs on
device in f32.

Two deliberate host-side preprocessing steps (both validated against the
f32 reference, gate is 2e-2):
- x is downcast to fp16 before upload: halves the HBM stream from 25.6MB
  to 12.8MB per core (measured rel err 6.3e-5).
- The 51 per-row score lookups (target + reversed tails; 13KB per core)
  are gathered on host from the same fp16 array the device streams, so
  the arithmetic is bit-identical to a device-side gather.  All three
  device gather mechanisms were measured on HW and share one serial Q7
  descriptor-generation path at ~10ns/element: 51 indirect DMAs = 85us,
  one dma_gather of 6656 window indices = 56us - either would dominate
  the ~58us exp stream, while the host gather is 0.1% of the input.

Scheduling notes (from NTFF traces of earlier revisions):
- Chunk-0's DMA is issued before the small uploads so the activation
  stream starts ASAP.
- log(total) is folded into the tail Ln activation as an extra input
  column, so the tail costs one Exp + one Ln (one act-table switch).
"""

import functools

import numpy as np

import concourse.bass as bass
import concourse.bacc as bacc
import concourse.tile as tile
from concourse import mybir
from concourse.bass_utils import run_bass_kernel_spmd

B = 1024
V = 50000
T = 50
M = 8            # cores
P = B // M       # 128 rows per core = SBUF partitions
# Chunk schedule: one small first chunk (own SBUF tile) so the scalar
# engine starts ~2us earlier, then uniform chunks rotating through a
# 3-deep pool (uniform size keeps the pool's buffer rotation valid -
# distinct sizes in one pool silently share a single buffer and
# serialize DMA behind compute).
C0 = 2036
CU = 6852
NCH = 7
assert C0 + NCH * CU == V
G = T + 1        # gathered scores per row: [target, reversed tails]

F16 = mybir.dt.float16
F32 = mybir.dt.float32


def _build_program() -> bass.Bass:
    nc = bacc.Bacc()
    x = nc.dram_tensor("x", [P, V], F16, kind="ExternalInput")
    sg = nc.dram_tensor("sg", [P, G], F16, kind="ExternalInput")
    maskr = nc.dram_tensor("maskr", [P, T], F32, kind="ExternalInput")
    loss = nc.dram_tensor("loss", [P, 1], F32, kind="ExternalOutput")

    with tile.TileContext(nc) as tc:
        with (
            tc.tile_pool(name="inp", bufs=3) as inp,
            tc.tile_pool(name="small", bufs=1) as small,
        ):
            # Chunk-0 (small, own tile) DMA first: the scalar engine starts
            # the exp stream earlier.  The two tiny tail uploads go next so
            # chunks 1-2 are issued right behind them.
            xt0 = small.tile([P, C0], F16)
            nc.sync.dma_start(out=xt0[:], in_=x[:, 0:C0])

            sg_t = small.tile([P, G], F16)
            nc.sync.dma_start(out=sg_t[:], in_=sg[:])
            maskr_t = small.tile([P, T], F32)
            nc.sync.dma_start(out=maskr_t[:], in_=maskr[:])

            # Funnel DMA-produced tiles through one DVE copy each so no
            # downstream instruction needs >1 cross-engine sync wait (the
            # TensorTensor encoding carries a single wait slot).  sg2 also
            # upcasts the gathered scores to f32 for the tail arithmetic.
            maskr2 = small.tile([P, T], F32)
            nc.vector.tensor_copy(out=maskr2[:], in_=maskr_t[:])
            sg2 = small.tile([P, G], F32)
            nc.vector.tensor_copy(out=sg2[:], in_=sg_t[:])

            # Main stream: total_exp[p] = sum_v exp(x[p, v]), chunked.  The
            # exp output itself is dead; one max-size scratch tile is reused
            # (consecutive ACTIVATEs are serial on Scalar anyway).
            sums = small.tile([P, NCH + 1], F32)
            et = small.tile([P, CU], F16)
            nc.scalar.activation(
                out=et[:, 0:C0],
                in_=xt0[:],
                func=mybir.ActivationFunctionType.Exp,
                accum_out=sums[:, 0:1],
            )
            for i in range(NCH):
                xt = inp.tile([P, CU], F16)
                nc.sync.dma_start(out=xt[:], in_=x[:, C0 + i * CU:C0 + (i + 1) * CU])
                nc.scalar.activation(
                    out=et[:],
                    in_=xt[:],
                    func=mybir.ActivationFunctionType.Exp,
                    accum_out=sums[:, i + 1:i + 2],
                )
            total = small.tile([P, 1], F32)
            nc.vector.reduce_sum(out=total[:], in_=sums[:], axis=mybir.AxisListType.X)

            # Tail term, all [P, <=52] f32 ops.  e_all has no artificial
            # ordering pin: its input is ready early, so wherever the
            # scheduler slots it on Scalar it reuses the loaded Exp table.
            e_all = small.tile([P, G], F32)
            nc.scalar.activation(
                out=e_all[:], in_=sg_t[:], func=mybir.ActivationFunctionType.Exp
            )
            es = small.tile([P, T], F32)
            nc.vector.tensor_mul(out=es[:], in0=e_all[:, 1:G], in1=maskr2[:])
            # lgin[:, 0:T] = cumsum(es) along t (== reference's cumsum of the
            # flipped masked exps); lgin[:, T] = total - others, so a single
            # Ln yields both the tail logs and log(total).
            lgin = small.tile([P, T + 1], F32)
            nc.vector.tensor_tensor_scan(
                out=lgin[:, 0:T],
                data0=es[:],
                data1=es[:],
                initial=0.0,
                op0=mybir.AluOpType.add,
                op1=mybir.AluOpType.bypass,
            )
            # others = total - exp(target_score) - sum(es); sum(es) = lgin[:, T-1]
            others = small.tile([P, 1], F32)
            nc.vector.tensor_scalar(
                out=others[:],
                in0=total[:],
                scalar1=e_all[:, 0:1],
                scalar2=lgin[:, T - 1:T],
                op0=mybir.AluOpType.subtract,
                op1=mybir.AluOpType.subtract,
            )
            nc.vector.tensor_scalar(
                out=lgin[:, T:T + 1],
                in0=total[:],
                scalar1=others[:],
                scalar2=None,
                op0=mybir.AluOpType.subtract,
            )
            # lg[:, t<T] = log(c_t + others); lg[:, T] = log(total)
            lg = small.tile([P, T + 1], F32)
            nc.scalar.activation(
                out=lg[:],
                in_=lgin[:],
                func=mybir.ActivationFunctionType.Ln,
                bias=others[:],
            )
            wl = small.tile([P, T], F32)
            nc.vector.tensor_mul(out=wl[:], in0=lg[:, 0:T], in1=maskr2[:])
            below = small.tile([P, 1], F32)
            nc.vector.reduce_sum(out=below[:], in_=wl[:], axis=mybir.AxisListType.X)
            sm = small.tile([P, T], F32)
            nc.vector.tensor_mul(out=sm[:], in0=sg2[:, 1:G], in1=maskr2[:])
            above = small.tile([P, 1], F32)
            nc.vector.reduce_sum(out=above[:], in_=sm[:], axis=mybir.AxisListType.X)

            # loss = -(target_score - log(total) + above - below)
            t1 = small.tile([P, 1], F32)
            nc.vector.tensor_scalar(
                out=t1[:],
                in0=lg[:, T:T + 1],
                scalar1=sg2[:, 0:1],
                scalar2=above[:],
                op0=mybir.AluOpType.subtract,
                op1=mybir.AluOpType.subtract,
            )
            res = small.tile([P, 1], F32)
            nc.vector.tensor_add(out=res[:], in0=t1[:], in1=below[:])
            nc.sync.dma_start(out=loss[:], in_=res[:])
    nc.finalize()  # runs the bacc passes (sync-wait splitting etc.)
    return nc


@functools.cache
def _program() -> bass.Bass:
    return _build_program()


def _prep_core_inputs(x16, target, tails, tail_len, core):
    r0 = core * P
    x = x16[r0:r0 + P]
    tgt = target[r0:r0 + P].astype(np.int64)
    tls = tails[r0:r0 + P].astype(np.int64)
    tln = tail_len[r0:r0 + P].astype(np.int64)

    # sg[p, 0] = x[p, target[p]]; sg[p, 1+t] = x[p, tails[p, T-1-t]]
    idx = np.concatenate([tgt[:, None], tls[:, ::-1]], axis=1)
    sg = np.take_along_axis(x, idx, axis=1)

    # maskr[r, t] = 1 iff reversed-tail position t is valid: (T-1-t) < tail_len[r]
    tpos = np.arange(T - 1, -1, -1, dtype=np.int64)[None, :]
    maskr = (tpos < tln[:, None]).astype(np.float32)
    return {
        "x": x,
        "sg": np.ascontiguousarray(sg),
        "maskr": np.ascontiguousarray(maskr),
    }


TRACE = False  # set by test.py for profiling runs; harness leaves it False


def kernel(output, target, tails, tail_len):
    output = np.asarray(output, dtype=np.float32)
    target = np.asarray(target)
    tails = np.asarray(tails)
    tail_len = np.asarray(tail_len)

    x16 = np.ascontiguousarray(output.astype(np.float16))
    in_maps = [
        _prep_core_inputs(x16, target, tails, tail_len, core) for core in range(M)
    ]
    out = run_bass_kernel_spmd(
        _program(), in_maps, core_ids=list(range(M)), trace=TRACE
    )
    global last_result
    last_result = out
    return np.concatenate(
        [r["loss"].reshape(P).astype(np.float32) for r in out.results]
    )


last_result = None


# revision 16
# speedup vs baseline: 1.8788x; 1.0039x over previous
"""ListMLE-with-tail loss kernel for Trainium2 (Bass/Tile), 8-core data-parallel.

Full-input contract: kernel(output[1024,50000] f32, target[1024] i32,
tails[1024,50] i32, tail_len[1024] i32) -> neg_like[1024] f32.

Sharding: batch rows split 128 per core (one row per SBUF partition).
Per core the dominant work - streaming the [128, 50000] row-slice through
the scalar engine's exp with fused per-chunk row-sum accumulation - runs
on device; the tail term (cumsum over <=50 exps + logs) also runs on
device in f32.

Two deliberate host-side preprocessing steps (both validated against the
f32 reference, gate is 2e-2):
- x is downcast to fp16 before upload: halves the HBM stream from 25.6MB
  to 12.8MB per core (measured rel err 6.3e-5).
- The 51 per-row score lookups (target + reversed tails; 13KB per core)
  are gathered on host from the same fp16 array the device streams, so
  the arithmetic is bit-identical to a device-side gather.  All three
  device gather mechanisms were measured on HW and share one serial Q7
  descriptor-generation path at ~10ns/element: 51 indirect DMAs = 85us,
  one dma_gather of 6656 window indices = 56us - either would dominate
  the ~58us exp stream, while the host gather is 0.1% of the input.

Scheduling notes (from NTFF traces of earlier revisions):
- Chunk-0's DMA is issued before the small uploads so the activation
  stream starts ASAP.
- log(total) is folded into the tail Ln activation as an extra input
  column, so the tail costs one Exp + one Ln (one act-table switch).
"""

import functools

import numpy as np

import concourse.bass as bass
import concourse.bacc as bacc
import concourse.tile as tile
from concourse import mybir
from concourse.bass_utils import run_bass_kernel_spmd

B = 1024
V = 50000
T = 50
M = 8            # cores
P = B // M       # 128 rows per core = SBUF partitions
# Chunk schedule: two small ramp chunks (own SBUF tiles) so the scalar
# engine starts ~2.5us earlier and chunk 1 arrives before chunk 0 is
# consumed, then uniform chunks rotating through a 3-deep pool (uniform
# size keeps the pool's buffer rotation valid - distinct sizes in one
# pool silently share a single buffer and serialize DMA behind compute).
C0 = 2040
C1 = 4080
CU = 8776
NCH = 5
assert C0 + C1 + NCH * CU == V
G = T + 1        # gathered scores per row: [target, reversed tails]

F16 = mybir.dt.float16
F32 = mybir.dt.float32


def _build_program() -> bass.Bass:
    nc = bacc.Bacc()
    x = nc.dram_tensor("x", [P, V], F16, kind="ExternalInput")
    sg = nc.dram_tensor("sg", [P, G], F16, kind="ExternalInput")
    maskr = nc.dram_tensor("maskr", [P, T], F32, kind="ExternalInput")
    loss = nc.dram_tensor("loss", [P, 1], F32, kind="ExternalOutput")

    with tile.TileContext(nc) as tc:
        with (
            tc.tile_pool(name="inp", bufs=3) as inp,
            tc.tile_pool(name="small", bufs=1) as small,
        ):
            # Ramp-chunk DMAs first: the scalar engine starts the exp stream
            # as early as possible.  The two tiny tail uploads go right
            # after; the uniform chunks are not needed until ~18us.
            xt0 = small.tile([P, C0], F16)
            nc.sync.dma_start(out=xt0[:], in_=x[:, 0:C0])
            xt1 = small.tile([P, C1], F16)
            nc.sync.dma_start(out=xt1[:], in_=x[:, C0:C0 + C1])

            sg_t = small.tile([P, G], F16)
            nc.sync.dma_start(out=sg_t[:], in_=sg[:])
            maskr_t = small.tile([P, T], F32)
            nc.sync.dma_start(out=maskr_t[:], in_=maskr[:])

            # Funnel DMA-produced tiles through one DVE copy each so no
            # downstream instruction needs >1 cross-engine sync wait (the
            # TensorTensor encoding carries a single wait slot).  sg2 also
            # upcasts the gathered scores to f32 for the tail arithmetic.
            maskr2 = small.tile([P, T], F32)
            nc.vector.tensor_copy(out=maskr2[:], in_=maskr_t[:])
            sg2 = small.tile([P, G], F32)
            nc.vector.tensor_copy(out=sg2[:], in_=sg_t[:])

            # Main stream: total_exp[p] = sum_v exp(x[p, v]), chunked.  The
            # exp output itself is dead; one max-size scratch tile is reused
            # (consecutive ACTIVATEs are serial on Scalar anyway).
            sums = small.tile([P, NCH + 2], F32)
            et = small.tile([P, CU], F16)
            nc.scalar.activation(
                out=et[:, 0:C0],
                in_=xt0[:],
                func=mybir.ActivationFunctionType.Exp,
                accum_out=sums[:, 0:1],
            )
            nc.scalar.activation(
                out=et[:, 0:C1],
                in_=xt1[:],
                func=mybir.ActivationFunctionType.Exp,
                accum_out=sums[:, 1:2],
            )
            base = C0 + C1
            for i in range(NCH):
                xt = inp.tile([P, CU], F16)
                nc.sync.dma_start(out=xt[:], in_=x[:, base + i * CU:base + (i + 1) * CU])
                nc.scalar.activation(
                    out=et[:],
                    in_=xt[:],
                    func=mybir.ActivationFunctionType.Exp,
                    accum_out=sums[:, i + 2:i + 3],
                )
            total = small.tile([P, 1], F32)
            nc.vector.reduce_sum(out=total[:], in_=sums[:], axis=mybir.AxisListType.X)

            # Tail term, all [P, <=52] f32 ops.  e_all has no artificial
            # ordering pin: its input is ready early, so wherever the
            # scheduler slots it on Scalar it reuses the loaded Exp table.
            e_all = small.tile([P, G], F32)
            nc.scalar.activation(
                out=e_all[:], in_=sg_t[:], func=mybir.ActivationFunctionType.Exp
            )
            es = small.tile([P, T], F32)
            nc.vector.tensor_mul(out=es[:], in0=e_all[:, 1:G], in1=maskr2[:])
            # lgin[:, 0:T] = cumsum(es) along t (== reference's cumsum of the
            # flipped masked exps); lgin[:, T] = total - others, so a single
            # Ln yields both the tail logs and log(total).
            lgin = small.tile([P, T + 1], F32)
            nc.vector.tensor_tensor_scan(
                out=lgin[:, 0:T],
                data0=es[:],
                data1=es[:],
                initial=0.0,
                op0=mybir.AluOpType.add,
                op1=mybir.AluOpType.bypass,
            )
            # others = total - exp(target_score) - sum(es); sum(es) = lgin[:, T-1]
            others = small.tile([P, 1], F32)
            nc.vector.tensor_scalar(
                out=others[:],
                in0=total[:],
                scalar1=e_all[:, 0:1],
                scalar2=lgin[:, T - 1:T],
                op0=mybir.AluOpType.subtract,
                op1=mybir.AluOpType.subtract,
            )
            nc.vector.tensor_scalar(
                out=lgin[:, T:T + 1],
                in0=total[:],
                scalar1=others[:],
                scalar2=None,
                op0=mybir.AluOpType.subtract,
            )
            # lg[:, t<T] = log(c_t + others); lg[:, T] = log(total)
            lg = small.tile([P, T + 1], F32)
            nc.scalar.activation(
                out=lg[:],
                in_=lgin[:],
                func=mybir.ActivationFunctionType.Ln,
                bias=others[:],
            )
            wl = small.tile([P, T], F32)
            nc.vector.tensor_mul(out=wl[:], in0=lg[:, 0:T], in1=maskr2[:])
            below = small.tile([P, 1], F32)
            nc.vector.reduce_sum(out=below[:], in_=wl[:], axis=mybir.AxisListType.X)
            sm = small.tile([P, T], F32)
            nc.vector.tensor_mul(out=sm[:], in0=sg2[:, 1:G], in1=maskr2[:])
            above = small.tile([P, 1], F32)
            nc.vector.reduce_sum(out=above[:], in_=sm[:], axis=mybir.AxisListType.X)

            # loss = -(target_score - log(total) + above - below)
            t1 = small.tile([P, 1], F32)
            nc.vector.tensor_scalar(
                out=t1[:],
                in0=lg[:, T:T + 1],
                scalar1=sg2[:, 0:1],
                scalar2=above[:],
                op0=mybir.AluOpType.subtract,
                op1=mybir.AluOpType.subtract,
            )
            res = small.tile([P, 1], F32)
            nc.vector.tensor_add(out=res[:], in0=t1[:], in1=below[:])
            nc.sync.dma_start(out=loss[:], in_=res[:])
    nc.finalize()  # runs the bacc passes (sync-wait splitting etc.)
    return nc


@functools.cache
def _program() -> bass.Bass:
    return _build_program()


def _prep_core_inputs(x16, target, tails, tail_len, core):
    r0 = core * P
    x = x16[r0:r0 + P]
    tgt = target[r0:r0 + P].astype(np.int64)
    tls = tails[r0:r0 + P].astype(np.int64)
    tln = tail_len[r0:r0 + P].astype(np.int64)

    # sg[p, 0] = x[p, target[p]]; sg[p, 1+t] = x[p, tails[p, T-1-t]]
    idx = np.concatenate([tgt[:, None], tls[:, ::-1]], axis=1)
    sg = np.take_along_axis(x, idx, axis=1)

    # maskr[r, t] = 1 iff reversed-tail position t is valid: (T-1-t) < tail_len[r]
    tpos = np.arange(T - 1, -1, -1, dtype=np.int64)[None, :]
    maskr = (tpos < tln[:, None]).astype(np.float32)
    return {
        "x": x,
        "sg": np.ascontiguousarray(sg),
        "maskr": np.ascontiguousarray(maskr),
    }


TRACE = False  # set by test.py for profiling runs; harness leaves it False


def kernel(output, target, tails, tail_len):
    output = np.asarray(output, dtype=np.float32)
    target = np.asarray(target)
    tails = np.asarray(tails)
    tail_len = np.asarray(tail_len)

    x16 = np.ascontiguousarray(output.astype(np.float16))
    in_maps = [
        _prep_core_inputs(x16, target, tails, tail_len, core) for core in range(M)
    ]
    out = run_bass_kernel_spmd(
        _program(), in_maps, core_ids=list(range(M)), trace=TRACE
    )
    global last_result
    last_result = out
    return np.concatenate(
        [r["loss"].reshape(P).astype(np.float32) for r in out.results]
    )


last_result = None


# revision 21
# speedup vs baseline: 2.3809x; 1.2673x over previous
"""ListMLE-with-tail loss kernel for Trainium2 (Bass/Tile), 8-core data-parallel.

Full-input contract: kernel(output[1024,50000] f32, target[1024] i32,
tails[1024,50] i32, tail_len[1024] i32) -> neg_like[1024] f32.

Sharding: batch rows split 128 per core (one row per SBUF partition).
Per core the dominant work - streaming the [128, 50000] row-slice through
the scalar engine's exp with fused per-chunk row-sum accumulation - runs
on device; the tail term (cumsum over <=50 exps + logs) also runs on
device in f32.

Two deliberate host-side preprocessing steps (both validated against the
f32 reference, gate is 2e-2):
- x is downcast to fp16 before upload: halves the HBM stream from 25.6MB
  to 12.8MB per core (measured rel err 6.3e-5).
- The 51 per-row score lookups (target + reversed tails; 13KB per core)
  are gathered on host from the same fp16 array the device streams, so
  the arithmetic is bit-identical to a device-side gather.  All three
  device gather mechanisms were measured on HW and share one serial Q7
  descriptor-generation path at ~10ns/element: 51 indirect DMAs = 85us,
  one dma_gather of 6656 window indices = 56us - either would dominate
  the ~58us exp stream, while the host gather is 0.1% of the input.

Scheduling notes (from NTFF traces of earlier revisions):
- Chunk-0's DMA is issued before the small uploads so the activation
  stream starts ASAP.
- log(total) is folded into the tail Ln activation as an extra input
  column, so the tail costs one Exp + one Ln (one act-table switch).
"""

import functools

import numpy as np

import concourse.bass as bass
import concourse.bacc as bacc
import concourse.tile as tile
from concourse import mybir
from concourse.bass_utils import run_bass_kernel_spmd

B = 1024
V = 50000
T = 50
M = 8            # cores
P = B // M       # 128 rows per core = SBUF partitions
# Uniform 6250-element chunks through a 3-deep pool measured fastest
# (71.77us): ramped first-chunk variants (2036/6852x7, 2040/4080/8776x5)
# measured 72.5-72.8us - the earlier activation start is eaten by queue
# serialization + the ~2us per-DMA completion receipt on the ramp chunks.
C = 6250
NCH = V // C
G = T + 1        # gathered scores per row: [target, reversed tails]

F16 = mybir.dt.float16
F32 = mybir.dt.float32


def _build_program() -> bass.Bass:
    nc = bacc.Bacc()
    x = nc.dram_tensor("x", [P, V], F16, kind="ExternalInput")
    sg = nc.dram_tensor("sg", [P, G], F16, kind="ExternalInput")
    maskr = nc.dram_tensor("maskr", [P, T], F32, kind="ExternalInput")
    loss = nc.dram_tensor("loss", [P, 1], F32, kind="ExternalOutput")

    with tile.TileContext(nc) as tc:
        with (
            tc.tile_pool(name="inp", bufs=3) as inp,
            tc.tile_pool(name="scratch", bufs=2) as scratch,
            tc.tile_pool(name="small", bufs=1) as small,
        ):
            # Chunk-0 DMA first: the scalar engine starts the exp stream
            # ~4us earlier than if the small uploads went first.
            xt0 = inp.tile([P, C], F16)
            nc.sync.dma_start(out=xt0[:], in_=x[:, 0:C])

            sg_t = small.tile([P, G], F16)
            nc.sync.dma_start(out=sg_t[:], in_=sg[:])
            maskr_t = small.tile([P, T], F32)
            nc.sync.dma_start(out=maskr_t[:], in_=maskr[:])

            # Funnel DMA-produced tiles through one DVE copy each so no
            # downstream instruction needs >1 cross-engine sync wait (the
            # TensorTensor encoding carries a single wait slot).  sg2 also
            # upcasts the gathered scores to f32 for the tail arithmetic.
            maskr2 = small.tile([P, T], F32)
            nc.vector.tensor_copy(out=maskr2[:], in_=maskr_t[:])
            sg2 = small.tile([P, G], F32)
            nc.vector.tensor_copy(out=sg2[:], in_=sg_t[:])

            # Main stream: total_exp[p] = sum_v exp(x[p, v]), chunked.  The
            # exp output itself is dead; one max-size scratch tile is reused
            # (consecutive ACTIVATEs are serial on Scalar anyway).
            sums = small.tile([P, NCH], F32)
            et0 = scratch.tile([P, C], F16, tag="exp_scratch")
            nc.scalar.activation(
                out=et0[:],
                in_=xt0[:],
                func=mybir.ActivationFunctionType.Exp,
                accum_out=sums[:, 0:1],
            )
            for i in range(1, NCH):
                xt = inp.tile([P, C], F16)
                nc.sync.dma_start(out=xt[:], in_=x[:, i * C:(i + 1) * C])
                et = scratch.tile([P, C], F16, tag="exp_scratch")
                nc.scalar.activation(
                    out=et[:],
                    in_=xt[:],
                    func=mybir.ActivationFunctionType.Exp,
                    accum_out=sums[:, i:i + 1],
                )
            total = small.tile([P, 1], F32)
            nc.vector.reduce_sum(out=total[:], in_=sums[:], axis=mybir.AxisListType.X)

            # Tail term, all [P, <=52] f32 ops.  e_all has no artificial
            # ordering pin: its input is ready early, so wherever the
            # scheduler slots it on Scalar it reuses the loaded Exp table.
            e_all = small.tile([P, G], F32)
            nc.scalar.activation(
                out=e_all[:], in_=sg_t[:], func=mybir.ActivationFunctionType.Exp
            )
            es = small.tile([P, T], F32)
            nc.vector.tensor_mul(out=es[:], in0=e_all[:, 1:G], in1=maskr2[:])
            # lgin[:, 0:T] = cumsum(es) along t (== reference's cumsum of the
            # flipped masked exps); lgin[:, T] = total - others, so a single
            # Ln yields both the tail logs and log(total).
            lgin = small.tile([P, T + 1], F32)
            nc.vector.tensor_tensor_scan(
                out=lgin[:, 0:T],
                data0=es[:],
                data1=es[:],
                initial=0.0,
                op0=mybir.AluOpType.add,
                op1=mybir.AluOpType.bypass,
            )
            # others = total - exp(target_score) - sum(es); sum(es) = lgin[:, T-1]
            others = small.tile([P, 1], F32)
            nc.vector.tensor_scalar(
                out=others[:],
                in0=total[:],
                scalar1=e_all[:, 0:1],
                scalar2=lgin[:, T - 1:T],
                op0=mybir.AluOpType.subtract,
                op1=mybir.AluOpType.subtract,
            )
            nc.vector.tensor_scalar(
                out=lgin[:, T:T + 1],
                in0=total[:],
                scalar1=others[:],
                scalar2=None,
                op0=mybir.AluOpType.subtract,
            )
            # lg[:, t<T] = log(c_t + others); lg[:, T] = log(total)
            lg = small.tile([P, T + 1], F32)
            nc.scalar.activation(
                out=lg[:],
                in_=lgin[:],
                func=mybir.ActivationFunctionType.Ln,
                bias=others[:],
            )
            wl = small.tile([P, T], F32)
            nc.vector.tensor_mul(out=wl[:], in0=lg[:, 0:T], in1=maskr2[:])
            below = small.tile([P, 1], F32)
            nc.vector.reduce_sum(out=below[:], in_=wl[:], axis=mybir.AxisListType.X)
            sm = small.tile([P, T], F32)
            nc.vector.tensor_mul(out=sm[:], in0=sg2[:, 1:G], in1=maskr2[:])
            above = small.tile([P, 1], F32)
            nc.vector.reduce_sum(out=above[:], in_=sm[:], axis=mybir.AxisListType.X)

            # loss = -(target_score - log(total) + above - below)
            t1 = small.tile([P, 1], F32)
            nc.vector.tensor_scalar(
                out=t1[:],
                in0=lg[:, T:T + 1],
                scalar1=sg2[:, 0:1],
                scalar2=above[:],
                op0=mybir.AluOpType.subtract,
                op1=mybir.AluOpType.subtract,
            )
            res = small.tile([P, 1], F32)
            nc.vector.tensor_add(out=res[:], in0=t1[:], in1=below[:])
            nc.sync.dma_start(out=loss[:], in_=res[:])
    nc.finalize()  # runs the bacc passes (sync-wait splitting etc.)
    return nc


@functools.cache
def _program() -> bass.Bass:
    return _build_program()


def _prep_core_inputs(x16, target, tails, tail_len, core):
    r0 = core * P
    x = x16[r0:r0 + P]
    tgt = target[r0:r0 + P].astype(np.int64)
    tls = tails[r0:r0 + P].astype(np.int64)
    tln = tail_len[r0:r0 + P].astype(np.int64)

    # sg[p, 0] = x[p, target[p]]; sg[p, 1+t] = x[p, tails[p, T-1-t]]
    idx = np.concatenate([tgt[:, None], tls[:, ::-1]], axis=1)
    sg = np.take_along_axis(x, idx, axis=1)

    # maskr[r, t] = 1 iff reversed-tail position t is valid: (T-1-t) < tail_len[r]
    tpos = np.arange(T - 1, -1, -1, dtype=np.int64)[None, :]
    maskr = (tpos < tln[:, None]).astype(np.float32)
    return {
        "x": x,
        "sg": np.ascontiguousarray(sg),
        "maskr": np.ascontiguousarray(maskr),
    }


TRACE = False  # set by test.py for profiling runs; harness leaves it False


def kernel(output, target, tails, tail_len):
    output = np.asarray(output, dtype=np.float32)
    target = np.asarray(target)
    tails = np.asarray(tails)
    tail_len = np.asarray(tail_len)

    x16 = np.ascontiguousarray(output.astype(np.float16))
    in_maps = [
        _prep_core_inputs(x16, target, tails, tail_len, core) for core in range(M)
    ]
    out = run_bass_kernel_spmd(
        _program(), in_maps, core_ids=list(range(M)), trace=TRACE
    )
    global last_result
    last_result = out
    return np.concatenate(
        [r["loss"].reshape(P).astype(np.float32) for r in out.results]
    )


last_result = None
